# revision 42
# baseline (speedup 1.0000x reference)
"""Trainium2 Bass kernel for nn_LocalCrossAttention.

Sharding: 8 cores = 4 heads (o) x 2 batch-halves; host pre-transposes
inputs/weights, device does everything else, host concatenates outputs.

Per core (NB=32 batches, fp16 projection operands by default):
- Weights stream as 48 quarter-strips [128, 4, 512]; projections accumulate
  512-wide PSUM chunks (Q, K, then V so K-dependent work starts early).
- Each 512-chunk is PE-transposed (4x [rows,128] -> [128,rows]) into a
  2-bank PSUM staging tile, then two wide strided DVE copies assemble the
  d-major layouts QT [64(d), b, q], KT/VT [64(d), b, i, k].
- Softmax cluster (emitted between K and V so it overlaps V's DMA): per
  quad of 4 batches: 4 scoresT matmuls [ik,q] into one PSUM bank; one Exp;
  one block-diag-ones matmul for per-i key sums; den = 1/(8*(sum+1e-5))
  folding the 1/sqrt(D) scale; attnT = e * den (kept live per quad).
- Endgame after V: per quad, 4 PE transposes build V [128(ik), d] slices,
  4 col-group-tiled Z matmuls (attnT.T @ V) into one PSUM tile, one ACT
  copy into the output staging tile, output DMA in two halves.
Numerics: fp16 multiplies, fp32 PSUM accumulation and fp32 attention;
measured rel err ~9e-4 vs the fp32 reference (gate family is 2e-2).
"""

import os

import numpy as np
import ml_dtypes

import concourse.bass as bass
import concourse.mybir as mybir
import concourse.tile as tile
from concourse import bacc
from concourse.bass_utils import run_bass_kernel_spmd

# ---- projection matmul dtype knob -------------------------------------------
# "fp32" (exact, 4 cyc/row), "fp32r" (1 cyc/row at N>=512), "bf16" (1 cyc/row,
# half DMA bytes), "fp16" (1 cyc/row, half DMA bytes, ~2^-11 mantissa)
MM_MODE = os.environ.get("LCA_MM_MODE", "fp16")

_MODES = {
    "fp32": (mybir.dt.float32, np.float32),
    "fp32r": (mybir.dt.float32r, np.float32),
    "bf16": (mybir.dt.bfloat16, ml_dtypes.bfloat16),
    "fp16": (mybir.dt.float16, np.float16),
}
MM_DT, MM_NP = _MODES[MM_MODE]

TRACE = False  # set True from test harness to capture ntff profile
LAST_RESULT = {}

B, O, I, LQ, LS, D = 64, 4, 4, 32, 32, 64
F = 2048
NB = 32  # batches per core
P = 128
NF = F // P  # 16 f-chunks
f32 = mybir.dt.float32
FP = mybir.ActivationFunctionType


_NC_CACHE = {}


def build_nc():
    key = MM_MODE
    if key in _NC_CACHE:
        return _NC_CACHE[key]
    nc = bacc.Bacc(
        trn_type="TRN2", target_bir_lowering=False, debug=False, num_devices=8
    )
    decT = nc.dram_tensor("decT", [F, NB], MM_DT, kind="ExternalInput").ap()
    encT = nc.dram_tensor("encT", [F, NB * I], MM_DT, kind="ExternalInput").ap()
    wqT = nc.dram_tensor("wqT", [F, F], MM_DT, kind="ExternalInput").ap()
    wkT = nc.dram_tensor("wkT", [F, F], MM_DT, kind="ExternalInput").ap()
    wvT = nc.dram_tensor("wvT", [F, F], MM_DT, kind="ExternalInput").ap()
    ident = nc.dram_tensor("ident", [P, P], f32, kind="ExternalInput").ap()
    bones = nc.dram_tensor("bones", [P, P], f32, kind="ExternalInput").ap()
    out = nc.dram_tensor("out", [NB, LQ, D], f32, kind="ExternalOutput").ap()

    with tile.TileContext(nc) as tc:
        with tc.tile_pool(name="persist", bufs=1) as persist:
            # activations first on the fast sync queue (small; the projections
            # need them immediately); late-needed consts on the gpsimd queue
            sb_decT = persist.tile([P, NF, NB], MM_DT)
            nc.sync.dma_start(
                out=sb_decT, in_=decT.rearrange("(fo fi) n -> fi fo n", fi=P)
            )
            sb_encT = persist.tile([P, NF, NB * I], MM_DT)
            nc.sync.dma_start(
                out=sb_encT, in_=encT.rearrange("(fo fi) n -> fi fo n", fi=P)
            )
            sb_ident = persist.tile([P, P], f32)
            nc.gpsimd.dma_start(out=sb_ident, in_=ident)
            sb_bones = persist.tile([P, P], f32)
            nc.gpsimd.dma_start(out=sb_bones, in_=bones)
            osb_all = persist.tile([P, NB // 4, D], f32)
            QT = persist.tile([64, NB, LQ], f32)
            KT = persist.tile([64, NB, I, LS], f32)
            VT = persist.tile([64, NB, I, LS], f32)
            Vsb = persist.tile([P, NB, D], f32)

            # ---- phase 1: projections --------------------------------------
            # Weight streamed as 4 column-strips [128, 16, 512]; per-512-chunk
            # PSUM accumulation; per-half-weight transpose-psum [128, 8, rows]
            # drained by 2 wide strided assembles into the d-major layouts.
            # PSUM bank plan (8 banks): sc(2) + bs(2) outer, psA(2) + psT(2)
            # scoped to the projections, vz(2) reuses the released psA zone
            # (its z/vt tiles depend on V anyway).
            _phases = os.environ.get("LCA_PHASES", "all")
            NQD = NB // 4
            with tc.tile_pool(name="attns", bufs=3) as attns, \
                 tc.tile_pool(name="atp", bufs=NQD) as atp, \
                 tc.tile_pool(name="outp", bufs=3) as outp, \
                 tc.tile_pool(name="wst", bufs=10) as wst, \
                 tc.tile_pool(name="praw", bufs=3) as praw, \
                 tc.tile_pool(name="psS", bufs=1, space="PSUM") as psS, \
                 tc.tile_pool(name="psB", bufs=1, space="PSUM") as psB:
                with tc.tile_pool(name="psA", bufs=3, space="PSUM") as psA, \
                     tc.tile_pool(name="psT", bufs=2, space="PSUM") as psT:

                    def project(wdram, xsb, rows, dest, is_q):
                        for m in range(4):
                            pst = psT.tile([P, 4, rows], f32, tag="tp")
                            t = m
                            halves = []
                            for hf in range(4):
                                sh = wst.tile([P, NF // 4, 512], MM_DT, tag="w")
                                nc.sync.dma_start(
                                    out=sh,
                                    in_=wdram[
                                        hf * F // 4:(hf + 1) * F // 4,
                                        t * 512:(t + 1) * 512,
                                    ].rearrange("(fo fi) n -> fi fo n", fi=P),
                                )
                                halves.append(sh)
                            ps = psA.tile([rows, 512], f32, tag="proj")
                            for f in range(NF):
                                nc.tensor.matmul(
                                    ps,
                                    lhsT=xsb[:, f],
                                    rhs=halves[f // (NF // 4)][:, f % (NF // 4)],
                                    start=(f == 0),
                                    stop=(f == NF - 1),
                                )
                            raw = praw.tile([rows, 512], f32, tag="raw")
                            nc.vector.tensor_copy(out=raw, in_=ps)
                            for j in range(4):
                                nc.tensor.transpose(
                                    pst[:, j, :],
                                    raw[:, j * P:(j + 1) * P],
                                    sb_ident[:rows, :rows],
                                )
                            for half in range(2):
                                src = pst[64 * half:64 * (half + 1)]
                                ksl = slice(8 * m + half, 8 * (m + 1), 2)
                                if is_q:
                                    nc.vector.tensor_copy(
                                        out=dest[:, :, ksl].rearrange(
                                            "d b q -> d q b"
                                        ),
                                        in_=src,
                                    )
                                else:
                                    nc.vector.tensor_copy(
                                        out=dest[:, :, :, ksl].rearrange(
                                            "d b i k -> d k b i"
                                        ),
                                        in_=src.rearrange(
                                            "d c (b i) -> d c b i", i=I
                                        ),
                                    )

                    project(wqT, sb_decT, NB, QT, True)
                    project(wkT, sb_encT, NB * I, KT, False)

                    # softmax cluster: emitted before V so it overlaps V's
                    # projection (priority order); at4 tiles stay live for z
                    at4s = []
                    for qd in range(NQD if _phases == "all" else 0):
                        sc4 = psS.tile([P, 4 * LQ], f32, tag="sc")
                        for j in range(4):
                            b = qd * 4 + j
                            nc.tensor.matmul(
                                sc4[:, 32 * j:32 * (j + 1)],
                                lhsT=KT[:, b].rearrange("d i k -> d (i k)"),
                                rhs=QT[:, b],
                                start=True,
                                stop=True,
                            )
                        e4 = attns.tile([P, 4 * LQ], f32, tag="e")
                        nc.scalar.activation(out=e4, in_=sc4, func=FP.Exp)
                        bs4 = psB.tile([P, 4 * LQ], f32, tag="bs")
                        nc.tensor.matmul(
                            bs4, lhsT=sb_bones, rhs=e4, start=True, stop=True
                        )
                        den4 = attns.tile([P, 4 * LQ], f32, tag="den")
                        nc.vector.tensor_scalar_add(den4, bs4, 1e-5)
                        nc.vector.reciprocal(den4, den4)
                        at4 = atp.tile([P, 4 * LQ], f32, tag="at")
                        nc.gpsimd.tensor_mul(at4, e4, den4)
                        at4s.append(at4)

                    project(wvT, sb_encT, NB * I, VT, False)

                # ---- phase 2: V re-layout + z + out -------------------------
                with tc.tile_pool(name="psV", bufs=3, space="PSUM") as psV:
                    if _phases == "proj":
                        # ablation: just dump QT to out and stop
                        osb = outp.tile([P, D], f32, tag="o")
                        nc.vector.tensor_copy(out=osb[:64, :32], in_=QT[:, 0])
                        nc.sync.dma_start(
                            out=out[0:4].rearrange("bl q d -> (bl q) d"), in_=osb
                        )
                    for qd in range(NQD if _phases == "all" else 0):
                        vt4 = psV.tile([P, 4, D], f32, tag="vt")
                        for j in range(4):
                            b = qd * 4 + j
                            nc.tensor.transpose(
                                vt4[:, j],
                                VT[:, b].rearrange("d i k -> d (i k)"),
                                sb_ident[:64, :64],
                            )
                        nc.vector.tensor_copy(
                            out=Vsb[:, qd * 4:(qd + 1) * 4], in_=vt4
                        )
                        z4 = psV.tile([P, D], f32, tag="z")
                        for j in range(4):
                            b = qd * 4 + j
                            nc.tensor.matmul(
                                z4[32 * j:32 * (j + 1), :],
                                lhsT=at4s[qd][:, 32 * j:32 * (j + 1)],
                                rhs=Vsb[:, b],
                                start=True,
                                stop=True,
                                tile_position=(0, 32 * j),
                            )
                        nc.scalar.activation(
                            out=osb_all[:, qd, :], in_=z4, func=FP.Copy, scale=0.125
                        )
                        if qd in (3, 7) and _phases == "all":
                            hh = qd // 4
                            nc.sync.dma_start(
                                out=out.rearrange(
                                    "(hh qd bl) q d -> (bl q) hh qd d", bl=4, hh=2
                                )[:, hh],
                                in_=osb_all[:, hh * 4:(hh + 1) * 4, :],
                            )
    nc.compile()
    _NC_CACHE[key] = nc
    return nc


def make_core_inputs(dec, enc, WQ_w, WK_w, WV_w, core):
    o, h = divmod(core, 2)
    bs = slice(h * NB, (h + 1) * NB)
    dec_c = dec[bs, o]  # [NB, F]
    enc_c = enc[bs].reshape(NB * I, F)  # [NB*I, F]
    return {
        "decT": np.ascontiguousarray(dec_c.T).astype(MM_NP),
        "encT": np.ascontiguousarray(enc_c.T).astype(MM_NP),
        "wqT": np.ascontiguousarray(WQ_w[o].T).astype(MM_NP),
        "wkT": np.ascontiguousarray(WK_w[o].T).astype(MM_NP),
        "wvT": np.ascontiguousarray(WV_w[o].T).astype(MM_NP),
        "ident": np.eye(P, dtype=np.float32),
        "bones": np.kron(np.eye(I), np.ones((LS, LS))).astype(np.float32),
    }


def _numpy_reference(dec, enc, WQ_w, WQ_b, WK_w, WK_b, WV_w, WV_b):
    Q = (np.einsum("bof,oqf->boq", dec, WQ_w) + WQ_b[None]).reshape(B, O, LQ, D)
    Kp = (np.einsum("bif,okf->boik", enc, WK_w) + WK_b[:, None]).reshape(
        B, O, I, LS, D)
    Vp = (np.einsum("bif,okf->boik", enc, WV_w) + WV_b[:, None]).reshape(
        B, O, I, LS, D)
    s = np.einsum("boqd,boikd->boiqk", Q, Kp)
    e = np.exp(s)
    attn = e / (e.sum(-1, keepdims=True) + 1e-5)
    Z = 0.125 * np.einsum("boiqk,boikd->boiqd", attn, Vp)
    return Z.sum(2).reshape(B, O * LQ, D).astype(np.float32)


def kernel(basic_decoder_out, final_encoder_out, WQ_w, WQ_b, WK_w, WK_b,
           WV_w, WV_b):
    dec = np.asarray(basic_decoder_out, dtype=np.float32).reshape(B, O, F)
    enc = np.asarray(final_encoder_out, dtype=np.float32).reshape(B, I, F)
    WQ_w = np.asarray(WQ_w, dtype=np.float32)
    WK_w = np.asarray(WK_w, dtype=np.float32)
    WV_w = np.asarray(WV_w, dtype=np.float32)
    WQ_b = np.asarray(WQ_b, dtype=np.float32)
    WK_b = np.asarray(WK_b, dtype=np.float32)
    WV_b = np.asarray(WV_b, dtype=np.float32)
    if WQ_b.any() or WK_b.any() or WV_b.any():
        # spec guarantees zero biases; host fallback just in case
        return _numpy_reference(dec, enc, WQ_w, WQ_b, WK_w, WK_b, WV_w, WV_b)

    nc = build_nc()
    in_maps = [
        make_core_inputs(dec, enc, WQ_w, WK_w, WV_w, c) for c in range(8)
    ]
    res = run_bass_kernel_spmd(nc, in_maps, core_ids=list(range(8)), trace=TRACE)
    LAST_RESULT["exec_time_ns"] = res.exec_time_ns
    LAST_RESULT["trace"] = res.instructions_and_trace

    full = np.zeros((B, O, LQ, D), dtype=np.float32)
    for c in range(8):
        o, h = divmod(c, 2)
        full[h * NB:(h + 1) * NB, o] = res.results[c]["out"]
    return full.reshape(B, O * LQ, D)


# ---- numpy mini-reference for a single core (selftest) ----------------------
def _core_expected(dec, enc, WQ_w, WK_w, WV_w, core):
    o, h = divmod(core, 2)
    bs = slice(h * NB, (h + 1) * NB)
    dec_c = dec[bs, o]  # [NB, F]
    enc_c = enc[bs]  # [NB, I, F]
    Q = (dec_c @ WQ_w[o].T).reshape(NB, LQ, D)
    K = np.einsum("bif,kf->bik", enc_c, WK_w[o]).reshape(NB, I, LS, D)
    V = np.einsum("bif,kf->bik", enc_c, WV_w[o]).reshape(NB, I, LS, D)
    scores = np.einsum("bqd,bikd->biqk", Q, K)
    e = np.exp(scores)
    attn = e / (e.sum(-1, keepdims=True) + 1e-5)
    Z = 0.125 * np.einsum("biqk,bikd->biqd", attn, V)
    return Z.sum(1)  # [NB, LQ, D]


def timeline(save_path=None):
    from concourse.timeline_sim import TimelineSim

    nc = build_nc()
    tl = TimelineSim(nc, trace=bool(save_path))
    total = tl.simulate()
    print(f"TimelineSim total: {total:.0f} ns")
    if save_path:
        tl.perfetto.save(save_path)
        print("saved", save_path)
    return total


if __name__ == "__main__":
    import os, sys

    rng = np.random.default_rng(0)
    dec = rng.standard_normal((B, O, F), dtype=np.float32)
    enc = rng.standard_normal((B, I, F), dtype=np.float32)
    WQ = (0.02 * rng.standard_normal((O, F, F))).astype(np.float32)
    WK = (0.02 * rng.standard_normal((O, F, F))).astype(np.float32)
    WV = (0.02 * rng.standard_normal((O, F, F))).astype(np.float32)

    if os.environ.get("TL"):
        timeline(os.environ.get("TLTRACE"))
        sys.exit(0)

    if os.environ.get("SIM"):
        from concourse.bass_interp import CoreSim

        nc = build_nc()
        sim = CoreSim(nc)
        ins = make_core_inputs(dec, enc, WQ, WK, WV, core=0)
        for k, v in ins.items():
            sim.tensor(k)[:] = v
        sim.simulate()
        got = np.asarray(sim.tensor("out"))
        exp = _core_expected(dec, enc, WQ, WK, WV, core=0)
        err = np.abs(got - exp)
        print("SIM absmax err:", err.max(), "rel:",
              np.linalg.norm(got - exp) / np.linalg.norm(exp))
    else:
        out = kernel(
            dec.reshape(B, O * LQ, D), enc.reshape(B, I * LS, D),
            WQ, np.zeros((O, F), np.float32),
            WK, np.zeros((O, F), np.float32),
            WV, np.zeros((O, F), np.float32),
        )
        exp = np.zeros((B, O, LQ, D), np.float32)
        for c in range(8):
            o, h = divmod(c, 2)
            exp[h * NB:(h + 1) * NB, o] = _core_expected(dec, enc, WQ, WK, WV, c)
        exp = exp.reshape(B, O * LQ, D)
        rel = np.linalg.norm(out - exp) / np.linalg.norm(exp)
        print("HW absmax err:", np.abs(out - exp).max(), "rel:", rel)
        sys.exit(0 if rel < 1e-4 else 1)
